# revision 1
# baseline (speedup 1.0000x reference)
"""Block self-attention (Gaussian kernel weights) Trainium2 Bass kernel, v3.

For each independent block of B=1024 rows of `features` [262144, 128]:
    w = exp(-(sq_i + sq_j - 2 x@x^T)/25.6);  out = (w @ x)/B
Blocks are data-parallel across 8 NeuronCores (32 blocks per core).

Key algebra: w = D_e A D_e with A = exp(2G/25.6) symmetric, e = exp(-sq/25.6).
  out_j = (e_j/B) * sum_i A_ij * (e_i x_i)
The diagonal i=j term equals x_j/B exactly (exponents cancel in fp32), so no
diag masking and no separate x/B add is needed; only bf16 quantization of
A_jj/y_j (~0.1% rms) touches the dominant term.

exp work uses w-symmetry: A chunks (ci,cj) computed only for cj>=ci
(36 of 64 per block), packed into a trapezoid stream of 4608 fp32 in PSUM
(2x [128,1536] tiles = 6 banks) -> 3 big ACT instrs per block.  Lower
triangle chunks come from DMA-xbar transposes (sync queue ONLY — DMA on the
scalar queue corrupts) into a [128, tgt, src, 128] mirror tile.

PSUM (8 banks): Gpack 2x3 + trt (PE in-transpose staging) 1 + outT 1.
mm2 runs in two j-half passes so outT needs only one bank at a time.
"""

import math
import os

os.environ.setdefault("NEURON_RT_RESET_CORES", "1")

import numpy as np

import concourse.bass as bass
import concourse.tile as tile
from concourse import bacc, mybir
from concourse.bass_utils import run_bass_kernel_spmd
from concourse.masks import make_identity

N_TOTAL = 262144
D = 128
B = 1024
NCORES = 8
ROWS_PER_CORE = N_TOTAL // NCORES   # 32768
NB_FULL = ROWS_PER_CORE // B        # 32 blocks per core
C = B // 128                        # 8 row-chunks per block

F32 = mybir.dt.float32
BF16 = mybir.dt.bfloat16
FP16 = mybir.dt.float16

SIGMA2X2 = 2.0 * (D / 10.0)         # 25.6
G_SCALE = 2.0 / SIGMA2X2            # 0.078125
NEG_INV = -1.0 / SIGMA2X2           # -0.0390625
# outT is cast fp32->fp16 with a 1/OSC scale to keep away from fp16 max;
# the tail multiplies by e_j*OSC/B.
OSC = 16.0

EXP = mybir.ActivationFunctionType.Exp
MULT = mybir.AluOpType.mult

# trapezoid packing: row c covers cols [128c, 1024) => width 1024-128c
ROW_W = [B - 128 * c for c in range(C)]
CUM = [0]
for w in ROW_W:
    CUM.append(CUM[-1] + w)
PACK = CUM[C]                        # 4608
TILE_W = 1536
NT = PACK // TILE_W                  # 3 ACT tiles per block


def mm1_pieces():
    """(tile_idx, off_in_tile, row_c, xt_col_start, n) split at 512 banks."""
    ps = []
    for c in range(C):
        s = CUM[c]
        while s < CUM[c + 1]:
            e = min(CUM[c + 1], (s // 512 + 1) * 512)
            ps.append((s // TILE_W, s % TILE_W, c, 128 * c + (s - CUM[c]), e - s))
            s = e
    return ps


MM1_PIECES = mm1_pieces()            # 15 MMs
MM1_BY_TILE = [[p for p in MM1_PIECES if p[0] == t] for t in range(NT)]


def mm2_half_pieces(h):
    """MM pieces for j in [512h, 512h+512): list of (c, kind, js, je) in
    emission order, with per-piece (start, stop) accumulation flags."""
    lo, hi = 512 * h, 512 * h + 512
    out = []
    for c in range(C):
        # mirror: j in [0, 128c); direct: j in [128c, 1024)
        mjs, mje = max(0, lo), min(128 * c, hi)
        if mje > mjs:
            out.append((c, "mir", mjs, mje))
        djs, dje = max(128 * c, lo), min(B, hi)
        if dje > djs:
            out.append((c, "dir", djs, dje))
    flags = [(i == 0, i == len(out) - 1) for i in range(len(out))]
    return list(zip(out, flags))


MM2_HALF = [mm2_half_pieces(0), mm2_half_pieces(1)]


def build(nb: int = NB_FULL) -> bacc.Bacc:
    rows = nb * B
    nc = bacc.Bacc("TRN2", target_bir_lowering=False, debug=False)

    fin = nc.dram_tensor("features", [rows, D], F32, kind="ExternalInput").ap()
    fout = nc.dram_tensor("out", [rows, D], F32, kind="ExternalOutput").ap()

    # row index = b*1024 + c*128 + p
    fin_v = fin.rearrange("(b c p) d -> b p c d", p=128, c=C)
    fout_v = fout.rearrange("(b c p) d -> b p c d", p=128, c=C)

    with tile.TileContext(nc) as tc:
        with (
            tc.tile_pool(name="const", bufs=1) as cpool,
            tc.tile_pool(name="xr", bufs=3) as xrpool,
            tc.tile_pool(name="xt", bufs=2) as xtpool,
            tc.tile_pool(name="y", bufs=5) as ypool,
            tc.tile_pool(name="sq", bufs=4) as sqpool,
            tc.tile_pool(name="ap", bufs=3) as apool,    # packed A [128,4608] bf16
            tc.tile_pool(name="m2", bufs=3) as m2pool,   # mirror [128,8,8,128] bf16
            tc.tile_pool(name="ot", bufs=2) as otpool,   # outT_sb fp16
            tc.tile_pool(name="tr", bufs=2) as trpool,   # trd fp16
            tc.tile_pool(name="of", bufs=2) as ofpool,   # out_final fp32
            tc.tile_pool(name="gp", bufs=2, space="PSUM") as gpool,
            tc.tile_pool(name="tp", bufs=1, space="PSUM") as tpool,
            tc.tile_pool(name="acc", bufs=1, space="PSUM") as accpool,
        ):
            identb = cpool.tile([128, 128], BF16)
            make_identity(nc, identb[:])

            state: dict[int, dict] = {}

            def load(b):
                xr = xrpool.tile([128, C, D], BF16)
                nc.gpsimd.dma_start(out=xr[:], in_=fin_v[b])  # SWDGE cast DMA
                state[b] = dict(xr=xr)

            def prep(b):
                st = state[b]
                xr = st["xr"]
                xsq = sqpool.tile([128, C * D], BF16, tag="xsq")
                nc.gpsimd.tensor_mul(
                    xsq[:], xr[:].rearrange("p c d -> p (c d)"),
                    xr[:].rearrange("p c d -> p (c d)"),
                )
                sqcol = sqpool.tile([128, C], F32, tag="sqc")
                nc.vector.tensor_reduce(
                    sqcol[:], xsq[:].rearrange("p (c d) -> p c d", d=D),
                    axis=mybir.AxisListType.X, op=mybir.AluOpType.add,
                )
                bias_col = sqpool.tile([128, C], F32, tag="bia")
                nc.vector.tensor_scalar_mul(bias_col[:], sqcol[:], NEG_INV)
                st["bias_col"] = bias_col

            def escalc(b):
                # escB[p,c,d] = exp(-sq[p,c]/25.6): one ACT instr on a
                # stride-0-broadcast input, fp16 out
                st = state[b]
                escB = ypool.tile([128, C, D], FP16, tag="escB")
                nc.scalar.activation(
                    escB[:],
                    st.pop("bias_col")[:].unsqueeze(2).broadcast_to([128, C, D]),
                    EXP,
                )
                st["escB"] = escB

            def ymul(b):
                st = state[b]
                y = ypool.tile([128, C, D], BF16, tag="y")
                nc.vector.tensor_mul(y[:], st["xr"][:], st["escB"][:])
                st["y"] = y

            def trans_in(b):
                st = state[b]
                trt = tpool.tile([128, C, 128], BF16, tag="trt")
                for c in range(C):
                    nc.tensor.transpose(
                        out=trt[:, c, :], in_=st["xr"][:, c, :], identity=identb[:]
                    )
                st["trt"] = trt

            def xt_copy(b):
                st = state[b]
                xT = xtpool.tile([128, C, 128], BF16)
                nc.vector.tensor_copy(
                    xT[:].rearrange("p c d -> p (c d)"),
                    st.pop("trt")[:].rearrange("p c d -> p (c d)"),
                )
                st["xT"] = xT

            def m1_tile(b, t):
                st = state[b]
                if t == 0:
                    st["g"] = {}
                    st["apk"] = apool.tile([128, PACK], BF16, name="apk", tag="apk")
                g = gpool.tile([128, TILE_W], F32, tag="g")
                st["g"][t] = g
                xT = st["xT"][:].rearrange("p c d -> p (c d)")
                for (_, off, c, col, n) in MM1_BY_TILE[t]:
                    nc.tensor.matmul(
                        g[:, off:off + n],
                        lhsT=st["xT"][:, c, :],
                        rhs=xT[:, col:col + n],
                        start=True, stop=True,
                    )

            def act_tile(b, t):
                st = state[b]
                g = st["g"].pop(t)
                nc.scalar.activation(
                    st["apk"][:, t * TILE_W:(t + 1) * TILE_W], g[:], EXP,
                    scale=G_SCALE,
                )

            def mir(b, ci):
                st = state[b]
                if ci == 0:
                    st["m2"] = m2pool.tile([128, C, C, 128], BF16, name="m2", tag="m2")
                nc.sync.dma_start_transpose(
                    out=st["m2"][:, ci + 1:C, ci, :],
                    in_=st["apk"][:, CUM[ci] + 128:CUM[ci + 1]],
                )

            def mm2_half(b, h):
                st = state[b]
                if h == 0:
                    st["ot"] = otpool.tile([128, B], FP16, name="ot", tag="ot")
                o = accpool.tile([128, 512], F32, tag="o")
                st["o"] = o
                for (c, kind, js, je), (start, stop) in MM2_HALF[h]:
                    if kind == "mir":
                        rhs = st["m2"][:, c, js // 128:je // 128, :]
                        rhs = rhs.rearrange("p s d -> p (s d)")
                    else:
                        lo = CUM[c] + (js - 128 * c)
                        rhs = st["apk"][:, lo:lo + (je - js)]
                    nc.tensor.matmul(
                        o[:, js - 512 * h:je - 512 * h],
                        lhsT=st["y"][:, c, :],
                        rhs=rhs,
                        start=start, stop=stop,
                    )

            def cast_half(b, h):
                st = state[b]
                nc.vector.tensor_scalar_mul(
                    st["ot"][:, h * 512:(h + 1) * 512], st.pop("o")[:], 1.0 / OSC
                )

            def tout(b):
                st = state[b]
                trd = trpool.tile([128, C, 128], FP16)
                nc.sync.dma_start_transpose(out=trd[:], in_=st.pop("ot")[:])
                st["trd"] = trd

            def tail(b):
                # out = (trd * OSC/B) * e_j  (one fused STT)
                st = state[b]
                of = ofpool.tile([128, C, D], F32)
                nc.vector.scalar_tensor_tensor(
                    out=of[:], in0=st["trd"][:], scalar=float(OSC / B),
                    in1=st["escB"][:], op0=MULT, op1=MULT,
                )
                st["of"] = of

            def store(b):
                st = state.pop(b)
                nc.sync.dma_start(out=fout_v[b], in_=st["of"][:])

            # software pipeline: iteration k handles load(k), prep/trans(k-1),
            # m1/act/mir(k-2), mm2/epilogue(k-3)
            for k in range(nb + 4):
                bl, bp, bm, be = k, k - 1, k - 2, k - 4
                if bl < nb:
                    load(bl)
                if 0 <= bp < nb:
                    prep(bp)
                if 0 <= bm < nb:
                    m1_tile(bm, 0)
                    act_tile(bm, 0)
                if 0 <= be < nb:
                    mm2_half(be, 0)
                if 0 <= bm < nb:
                    mir(bm, 0)
                    m1_tile(bm, 1)
                    act_tile(bm, 1)
                    mir(bm, 1)
                    mir(bm, 2)
                if 0 <= be < nb:
                    cast_half(be, 0)
                if 0 <= bm < nb:
                    m1_tile(bm, 2)
                    act_tile(bm, 2)
                if 0 <= be < nb:
                    mm2_half(be, 1)
                    cast_half(be, 1)
                    tout(be)
                if 0 <= bm < nb:
                    for ci in range(3, 7):
                        mir(bm, ci)
                if 0 <= bp < nb:
                    trans_in(bp)
                    xt_copy(bp)
                    escalc(bp)
                    ymul(bp)
                if 0 <= be < nb:
                    tail(be)
                    store(be)

    nc.compile()
    return nc


_CACHE: dict[int, bacc.Bacc] = {}


def _get_nc(nb: int = NB_FULL) -> bacc.Bacc:
    if nb not in _CACHE:
        _CACHE[nb] = build(nb)
    return _CACHE[nb]


def run(features: np.ndarray, nc: bacc.Bacc | None = None, **spmd_kwargs):
    """Shard rows across 8 cores, run, gather. Returns (out, BassKernelResults)."""
    features = np.ascontiguousarray(features, dtype=np.float32)
    assert features.shape == (N_TOTAL, D)
    if nc is None:
        nc = _get_nc()
    core_ids = list(range(NCORES))
    shards = np.split(features, NCORES, axis=0)
    in_maps = [{"features": s} for s in shards]
    res = run_bass_kernel_spmd(nc, in_maps, core_ids, **spmd_kwargs)
    out = np.concatenate([res.results[i]["out"] for i in range(NCORES)], axis=0)
    return out, res


def kernel(features: np.ndarray) -> np.ndarray:
    out, _ = run(features)
    return out

